# revision 72
# baseline (speedup 1.0000x reference)
"""Trainium2 Bass kernel for a dense transformer block (B=8, S=512, D=768, H=12, Fd=3072).

Sharding: pure data-parallel over batch - one batch element per NeuronCore,
weights replicated, no collectives.

Key design points (v3):
  - T-layout attention (activations kept [feature, seq]) so the PE never
    transposes the attention matrix.
  - Host-side KEY COMPACTION: only the mask==1 key positions (~256 of 512,
    padded to KC=384) participate in scores / exp / attn@V.
  - The gaussian positional bias (+ -30 pad-kill) is ADDED into the scores
    PSUM via an identity-matmul (glog moving operand), so the softmax is a
    single Exp activation per chunk - no elementwise multiplies.
  - Softmax normalization: [v | 1] augmented V gives per-query sums free;
    sum row -> SBUF (DVE copy), reciprocal_approx_fast, gpsimd
    partition_broadcast (proxy ucode library), one DVE multiply.
  - LN1 emits the normalized row via one scalar-engine ACT (per-partition
    scale/bias); gamma1/beta1 are folded into w1/b1/b2 on host.
  - Weights th-major so the first q/k tiles need 1/6 of the weight DMA;
    DMA ring order defers w1/w2 behind the critical lead-in tensors.
  - FFN identical structure to v1 (it ran at 91-96% PE efficiency).

Matmul inputs are bf16 (weights pre-cast on host), PSUM accumulation f32.
"""

import os
import numpy as np
import ml_dtypes

import concourse.bass as bass
import concourse.mybir as mybir
import concourse.tile as tile
from concourse import bacc
from concourse import bass_utils
from concourse.masks import make_identity
from concourse import library_config

_DBG = bool(os.environ.get("KBENCH_DEBUG_DUMP"))

BF = mybir.dt.bfloat16
F32 = mybir.dt.float32
AF = mybir.ActivationFunctionType
OP = mybir.AluOpType

B, S, D, H, Dh, Fd = 8, 512, 768, 12, 64, 3072
NCORES = 8

KD = D // 128      # 6  K-tiles over D
MS = S // 128      # 4  chunks over sequence
KF = Fd // 128     # 24 K-tiles over Fd
NT = 2             # N-tiles over D for natural-layout outputs (2 x 384)
ND = D // NT       # 384
EPS = 1e-12


def _trace(nc, io, nkc):
    with tile.TileContext(nc) as tc:
        _trace_body(nc, tc, io, nkc)


def _trace_body(nc, tc, io, nkc):
    from contextlib import ExitStack

    KC = nkc * 128  # compacted key count

    with ExitStack() as ctx:
        glob = ctx.enter_context(tc.tile_pool(name="glob", bufs=1))

        # ---- constants / small inputs (gpsimd ring: smalls first) ----
        biasf_sb = glob.tile([128, 2 * KD + KF], F32, tag="biasf")
        nc.gpsimd.dma_start(out=biasf_sb, in_=io["bias_f"])
        bq_c = biasf_sb[:, 0:KD]
        bk_c = biasf_sb[:, KD : 2 * KD]
        b1_c = biasf_sb[:, 2 * KD : 2 * KD + KF]

        # biasb and gamma/beta rows packed into one single-partition row
        smalls2 = glob.tile([1, 7 * D], BF, tag="smalls2")
        nc.gpsimd.dma_start(out=smalls2, in_=io["rows2"])
        bv_r = smalls2[:, 0:D]
        bproj_r = smalls2[:, D : 2 * D]
        b2_r = smalls2[:, 2 * D : 3 * D]
        gb_row = smalls2[:, 3 * D : 7 * D]
        gbt = glob.tile([128, 4, D], BF, tag="gbt")
        g1b, be1b, g2b, be2b = gbt[:, 0, :], gbt[:, 1, :], gbt[:, 2, :], gbt[:, 3, :]

        ident_bf = glob.tile([128, 128], BF, tag="ident")
        make_identity(nc, ident_bf)
        ones_bf = glob.tile([1, 512], BF, tag="ones_bf")
        nc.vector.memset(ones_bf, 1.0)
        eps_t = glob.tile([128, 1], F32, tag="eps")
        nc.vector.memset(eps_t, EPS)


        # x and xT share one wide-row tile (single fast DMA)
        xxT_sb = glob.tile([128, MS * D + KD * S], BF, tag="xxT")
        x_sb = xxT_sb[:, 0 : MS * D].rearrange("p (c n) -> p c n", n=D)
        xT_sb = xxT_sb[:, MS * D :].rearrange("p (c s) -> p c s", s=S)
        hT_sb = glob.tile([128, KD, S], BF, tag="hT")
        h1b_sb = glob.tile([128, MS, D], BF, tag="h1b")
        h1T_sb = glob.tile([128, KD, S], BF, tag="h1T")
        w1_sb = glob.tile([128, KD, Fd], BF, tag="w1")
        # w2 at top level so it can stream in during attention
        w2_sb = glob.tile([128, KF, D], BF, tag="w2")

        last_mult = None

        # ================= attention scope =================
        with ExitStack() as attn_ctx:
            attnp = attn_ctx.enter_context(tc.tile_pool(name="attn", bufs=1))
            # sync ring: xTc first (critical)
            xTc_sb = attnp.tile([128, KD, KC], BF, tag="xTc")
            nc.sync.dma_start(
                out=xTc_sb.rearrange("p c s -> p (c s)"), in_=io["xTc"]
            )
            gexpc_sb = attnp.tile([128, nkc, S], BF, tag="gexpc")
            wv_sb = attnp.tile([128, KD, D], BF, tag="wv")
            wp_sb = attnp.tile([128, KD, D], BF, tag="wp")
            qT_sb = attnp.tile([128, KD, S], BF, tag="qT")
            kTc_sb = attnp.tile([128, KD, KC], BF, tag="kTc")
            vc_sb = attnp.tile([128, nkc, H, Dh + 1], BF, tag="vc")

            # ---- q/k projections in a short-lived weight pool ----
            psA_cm = tc.tile_pool(name="psA", bufs=1, space="PSUM")
            psA = psA_cm.__enter__()
            with tc.tile_pool(name="wqwk", bufs=1) as wqwk:
                # gpsimd ring (after smalls): wk th-major halves, gexpc
                wk_sb = wqwk.tile([128, KD, KD, 128], BF, tag="wk")
                nc.gpsimd.dma_start(
                    out=wk_sb[:, 0:3].rearrange("p a b c -> p (a b c)"),
                    in_=io["wk_bf"][:, 0 : 3 * KD * 128],
                )
                nc.gpsimd.dma_start(
                    out=wk_sb[:, 3:6].rearrange("p a b c -> p (a b c)"),
                    in_=io["wk_bf"][:, 3 * KD * 128 : 6 * KD * 128],
                )
                nc.gpsimd.dma_start(
                    out=gexpc_sb.rearrange("p c s -> p (c s)"), in_=io["gexpc"]
                )
                nc.gpsimd.load_library(library_config.proxy)
                nc.gpsimd.partition_broadcast(
                    gbt.rearrange("p c n -> p (c n)"), gb_row
                )
                # sync ring (after xTc): wq halves
                wq_sb = wqwk.tile([128, KD, KD, 128], BF, tag="wq")
                nc.sync.dma_start(
                    out=wq_sb[:, 0:3].rearrange("p a b c -> p (a b c)"),
                    in_=io["wq_bf"][:, 0 : 3 * KD * 128],
                )
                nc.sync.dma_start(
                    out=wq_sb[:, 3:6].rearrange("p a b c -> p (a b c)"),
                    in_=io["wq_bf"][:, 3 * KD * 128 : 6 * KD * 128],
                )
                # scalar ring: wv (for v right after kq), then x+xT, wp
                nc.scalar.dma_start(
                    out=wv_sb.rearrange("p c n -> p (c n)"), in_=io["wv_bf"]
                )
                nc.scalar.dma_start(out=xxT_sb, in_=io["xxT"])
                nc.scalar.dma_start(
                    out=wp_sb.rearrange("p c n -> p (c n)"), in_=io["wproj_bf"]
                )
                nc.vector.memset(vc_sb[:, :, :, Dh : Dh + 1], 1.0)

                def kq_tile(th, which):
                    if which == "k":
                        w, xx, n, dst, bias = wk_sb, xTc_sb, KC, kTc_sb, bk_c
                    else:
                        w, xx, n, dst, bias = wq_sb, xT_sb, S, qT_sb, bq_c
                    ps = psA.tile([128, 512], F32, tag="acc", bufs=3,
                                  name="ps_kq")
                    for k in range(KD):
                        nc.tensor.matmul(
                            ps[:, 0:n], w[:, th, k, :], xx[:, k, :],
                            start=(k == 0), stop=(k == KD - 1),
                        )
                    # PSUM->SBUF move + per-partition bias on DVE, keeping
                    # the scalar engine free for the exp stream
                    return nc.vector.tensor_scalar_add(
                        out=dst[:, th, :], in0=ps[:, 0:n],
                        scalar1=bias[:, th : th + 1],
                    )

                for th in range(3):
                    kq_tile(th, "k")
                for th in range(3):
                    kq_tile(th, "q")
                for th in range(3, KD):
                    kq_tile(th, "k")
                for th in range(3, KD):
                    kq_tile(th, "q")

            def v_tiles(n):
                for c in range(nkc):
                    ps = psA.tile([128, 512], F32, tag="acc", bufs=3, name="ps_v")
                    for k in range(KD):
                        nc.tensor.matmul(
                            ps[:, 0:ND], xTc_sb[:, k, 128 * c : 128 * (c + 1)],
                            wv_sb[:, k, ND * n : ND * (n + 1)],
                            start=(k == 0), stop=False,
                        )
                    nc.tensor.matmul(
                        ps[:, 0:ND], ones_bf[:, 0:128],
                        bv_r[:, ND * n : ND * (n + 1)],
                        start=False, stop=True,
                    )
                    nc.vector.tensor_copy(
                        out=vc_sb[:, c, 6 * n : 6 * (n + 1), 0:Dh],
                        in_=ps[:, 0:ND].rearrange("p (h d) -> p h d", d=Dh),
                    )

            # w1 / w2 transfers ride the rings behind the critical lead-in
            nc.scalar.dma_start(
                out=w1_sb[:, 0:3, :].rearrange("p c n -> p (c n)"),
                in_=io["w1_bf"][:, 0:3, :].rearrange("p c n -> p (c n)"),
            )
            nc.gpsimd.dma_start(
                out=w1_sb[:, 3:6, :].rearrange("p c n -> p (c n)"),
                in_=io["w1_bf"][:, 3:6, :].rearrange("p c n -> p (c n)"),
            )
            nc.scalar.dma_start(
                out=w2_sb[:, 0:12, :].rearrange("p c n -> p (c n)"),
                in_=io["w2_bf"][:, 0:12, :].rearrange("p c n -> p (c n)"),
            )
            nc.gpsimd.dma_start(
                out=w2_sb[:, 12:24, :].rearrange("p c n -> p (c n)"),
                in_=io["w2_bf"][:, 12:24, :].rearrange("p c n -> p (c n)"),
            )
            v_tiles(0)
            v_tiles(1)
            psA_cm.__exit__(None, None, None)

            if _DBG:
                nc.scalar.dma_start(
                    out=io["dbg_qT"], in_=qT_sb.rearrange("p a b -> p (a b)")
                )
                nc.scalar.dma_start(
                    out=io["dbg_kTc"], in_=kTc_sb.rearrange("p a b -> p (a b)")
                )

            # ---- heads scope ----
            with tc.tile_pool(name="headsp", bufs=1) as hp, \
                 tc.tile_pool(name="psS", bufs=1, space="PSUM") as psS, \
                 tc.tile_pool(name="psH", bufs=1, space="PSUM") as psH:

                def head(h):
                    th, off = h // 2, (h % 2) * 64
                    qh = qT_sb[off : off + 64, th, :]
                    exs = []
                    for c in range(nkc):
                        ps_sc = psS.tile([128, 512], F32, tag="sc", bufs=4,
                                         name="ps_sc")
                        nc.tensor.matmul(
                            ps_sc, kTc_sb[off : off + 64, th,
                                          128 * c : 128 * (c + 1)],
                            qh, start=True, stop=True,
                        )
                        ex = hp.tile([128, 512], BF, tag="ex", bufs=9, name="ex")
                        nc.scalar.activation(out=ex, in_=ps_sc, func=AF.Exp)
                        eng = nc.gpsimd if c == 2 else nc.vector
                        eng.tensor_tensor(
                            out=ex, in0=ex, in1=gexpc_sb[:, c, :], op=OP.mult
                        )
                        exs.append(ex)
                    ps_h = psH.tile([Dh + 1, 512], F32, tag="hT", bufs=4,
                                    name="ps_h")
                    for c in range(nkc):
                        nc.tensor.matmul(
                            ps_h, vc_sb[:, c, h, :], exs[c],
                            start=(c == 0), stop=(c == nkc - 1),
                        )
                    srow = hp.tile([1, 512], F32, tag="srow", bufs=3, name="srow")
                    if h % 2 == 0:
                        nc.scalar.copy(out=srow, in_=ps_h[Dh : Dh + 1, :])
                    else:
                        nc.vector.tensor_copy(out=srow, in_=ps_h[Dh : Dh + 1, :])
                    rec = hp.tile([1, 512], F32, tag="rec", bufs=3, name="rec")
                    nc.vector.reciprocal_approx_fast(out=rec, in_=srow)
                    bca = hp.tile([64, 512], F32, tag="bca", bufs=3, name="bca")
                    nc.gpsimd.partition_broadcast(bca, rec)
                    return nc.vector.tensor_tensor(
                        out=hT_sb[off : off + 64, th, :], in0=ps_h[0:Dh, :],
                        in1=bca, op=OP.mult,
                    )

                for h in range(H):
                    last_mult = head(h)

            # ---- proj + residual + LN1 + h1 transpose ----
            with tc.tile_pool(name="psP", bufs=1, space="PSUM") as psP, \
                 tc.tile_pool(name="psT", bufs=1, space="PSUM") as psT:
                for m in range(MS):
                    pss = []
                    for n in range(NT):
                        ps = psP.tile([128, 512], F32, tag="pacc", bufs=3,
                                      name="ps_pr")
                        for k in range(KD):
                            nc.tensor.matmul(
                                ps[:, 0:ND],
                                hT_sb[:, k, 128 * m : 128 * (m + 1)],
                                wp_sb[:, k, ND * n : ND * (n + 1)],
                                start=(k == 0), stop=False,
                            )
                        nc.tensor.matmul(
                            ps[:, 0:ND], ones_bf[:, 0:128],
                            bproj_r[:, ND * n : ND * (n + 1)],
                            start=False, stop=True,
                        )
                        pss.append(ps)
                    row = glob.tile([128, D], F32, tag="rowtmp", bufs=2, name="row")
                    nc.vector.tensor_tensor(
                        out=row[:, 0:ND], in0=pss[0][:, 0:ND],
                        in1=x_sb[:, m, 0:ND], op=OP.add,
                    )
                    nc.vector.tensor_tensor(
                        out=row[:, ND:D], in0=pss[1][:, 0:ND],
                        in1=x_sb[:, m, ND:D], op=OP.add,
                    )
                    _layernorm(nc, glob, row, g1b, be1b, eps_t, h1b_sb[:, m, :],
                               fold=True)
                    for f in range(KD):
                        ps_t = psT.tile([128, 128], BF, tag="tr", bufs=2,
                                        name="ps_t")
                        nc.tensor.transpose(
                            ps_t, h1b_sb[:, m, 128 * f : 128 * (f + 1)], ident_bf
                        )
                        if f % 2 == 0:
                            nc.scalar.copy(
                                out=h1T_sb[:, f, 128 * m : 128 * (m + 1)], in_=ps_t
                            )
                        else:
                            nc.vector.tensor_copy(
                                out=h1T_sb[:, f, 128 * m : 128 * (m + 1)], in_=ps_t
                            )

        if _DBG:
            nc.scalar.dma_start(
                out=io["dbg_hT"], in_=hT_sb.rearrange("p a b -> p (a b)")
            )
            nc.scalar.dma_start(
                out=io["dbg_h1b"], in_=h1b_sb.rearrange("p a b -> p (a b)")
            )
            nc.scalar.dma_start(
                out=io["dbg_h1T"], in_=h1T_sb.rearrange("p a b -> p (a b)")
            )

        # ================= FFN scope =================
        with tc.tile_pool(name="ffnp", bufs=1) as ffnp, \
             tc.tile_pool(name="psF", bufs=1, space="PSUM") as psF:
            ff1T_sb = ffnp.tile([128, KF, S], BF, tag="ff1T")
            for fm in range(KF):
                ps = psF.tile([128, 512], F32, tag="facc", bufs=8, name="ps_f1")
                for k in range(KD):
                    nc.tensor.matmul(
                        ps, w1_sb[:, k, 128 * fm : 128 * (fm + 1)], h1T_sb[:, k, :],
                        start=(k == 0), stop=(k == KD - 1),
                    )
                nc.scalar.activation(
                    out=ff1T_sb[:, fm, :], in_=ps, func=AF.Gelu,
                    bias=b1_c[:, fm : fm + 1], scale=1.0,
                )

            out_engines = (nc.sync, nc.gpsimd, nc.scalar)
            for m in range(MS):
                pss = []
                for n in range(NT):
                    ps = psF.tile([128, 512], F32, tag="facc", bufs=8, name="ps_f2")
                    for k in range(KF):
                        nc.tensor.matmul(
                            ps[:, 0:ND],
                            ff1T_sb[:, k, 128 * m : 128 * (m + 1)],
                            w2_sb[:, k, ND * n : ND * (n + 1)],
                            start=(k == 0), stop=False,
                        )
                    nc.tensor.matmul(
                        ps[:, 0:ND], ones_bf[:, 0:128],
                        b2_r[:, ND * n : ND * (n + 1)],
                        start=False, stop=True,
                    )
                    pss.append(ps)
                # residual = gamma1 * h1_hat (+ beta1 folded into b2 row)
                tmp = glob.tile([128, D], F32, tag="restmp", bufs=2, name="tmp")
                nc.vector.tensor_tensor(
                    out=tmp, in0=h1b_sb[:, m, :], in1=g1b, op=OP.mult
                )
                row = glob.tile([128, D], F32, tag="rowtmp", bufs=2, name="row2")
                nc.vector.tensor_tensor(
                    out=row[:, 0:ND], in0=pss[0][:, 0:ND],
                    in1=tmp[:, 0:ND], op=OP.add,
                )
                nc.vector.tensor_tensor(
                    out=row[:, ND:D], in0=pss[1][:, 0:ND],
                    in1=tmp[:, ND:D], op=OP.add,
                )
                outrow = glob.tile([128, D], BF, tag="outrow", bufs=2,
                                   name="outrow")
                _layernorm(nc, glob, row, g2b, be2b, eps_t, outrow)
                if m < MS - 1:
                    out_engines[m].dma_start(
                        out=io["out"][128 * m : 128 * (m + 1), :], in_=outrow
                    )
                else:
                    nc.sync.dma_start(
                        out=io["out"][128 * m : 128 * (m + 1), 0:ND],
                        in_=outrow[:, 0:ND],
                    )
                    nc.scalar.dma_start(
                        out=io["out"][128 * m : 128 * (m + 1), ND:D],
                        in_=outrow[:, ND:D],
                    )


def _layernorm(nc, pool, row, gamma_b, beta_b, eps_t, out_ap, fold=False):
    st = pool.tile([128, 3, 6], F32, tag="st", bufs=2, name="st")
    for g in range(3):
        nc.vector.bn_stats(out=st[:, g, :], in_=row[:, 256 * g : 256 * (g + 1)])
    mv = pool.tile([128, 2], F32, tag="mv", bufs=2, name="mv")
    nc.vector.bn_aggr(out=mv, in_=st)
    sd = pool.tile([128, 1], F32, tag="sd", bufs=2, name="sd")
    nc.scalar.activation(out=sd, in_=mv[:, 1:2], func=AF.Sqrt, bias=eps_t, scale=1.0)
    rs = pool.tile([128, 1], F32, tag="rs", bufs=2, name="rs")
    nc.vector.reciprocal(rs, sd)
    if fold:
        # gamma/beta folded into downstream weights on host: emit the
        # normalized row via one ACT (per-partition scale rstd, bias -mu*rstd)
        nb = pool.tile([128, 1], F32, tag="nb", bufs=2, name="nb")
        nc.vector.scalar_tensor_tensor(
            out=nb, in0=mv[:, 0:1], scalar=-1.0, in1=rs,
            op0=OP.mult, op1=OP.mult,
        )
        nc.scalar.activation(
            out=out_ap, in_=row, func=AF.Identity, bias=nb, scale=rs
        )
        return
    # in-place: row = (row - mean) * gamma ; out = row * rstd + beta
    nc.vector.scalar_tensor_tensor(
        out=row, in0=row, scalar=mv[:, 0:1], in1=gamma_b,
        op0=OP.subtract, op1=OP.mult,
    )
    nc.vector.scalar_tensor_tensor(
        out=out_ap, in0=row, scalar=rs, in1=beta_b,
        op0=OP.mult, op1=OP.add,
    )


def _specs(nkc):
    KC = nkc * 128
    return [
        ("xxT", [128, MS * D + KD * S], BF),   # x | xT packed (wide rows)
        ("xTc", [128, KD * KC], BF),
        ("gexpc", [128, nkc * S], BF),
        ("wq_bf", [128, KD * KD * 128], BF),   # th-major [p, th, k, c]
        ("wk_bf", [128, KD * KD * 128], BF),   # th-major
        ("wv_bf", [128, KD * D], BF),
        ("wproj_bf", [128, KD * D], BF),
        ("w1_bf", [128, KD, Fd], BF),
        ("w2_bf", [128, KF, D], BF),
        ("bias_f", [128, 2 * KD + KF], F32),   # bq*0.125 | bk | b1' pcols
        ("rows2", [1, 7 * D], BF),             # bv|bproj|b2'|gamma1|beta1|gamma2|beta2
    ]


_BUILT = {}


def _build(nkc):
    if nkc in _BUILT:
        return _BUILT[nkc]
    nc = bacc.Bacc("TRN2", target_bir_lowering=False, debug=False,
                   enable_asserts=False, num_devices=NCORES)
    io = {}
    for name, shape, dt in _specs(nkc):
        io[name] = nc.dram_tensor(name, shape, dt, kind="ExternalInput").ap()
    io["out"] = nc.dram_tensor("out", [S, D], BF, kind="ExternalOutput").ap()
    io["bscratch"] = nc.dram_tensor("bscratch", [3, 4, 512], F32,
                                    kind="Internal").ap()
    if _DBG:
        for nm, shape in [
            ("dbg_qT", [128, KD * S]), ("dbg_kTc", [128, KD * nkc * 128]),
            ("dbg_hT", [128, KD * S]),
            ("dbg_h1b", [128, MS * D]), ("dbg_h1T", [128, KD * S]),
        ]:
            io[nm] = nc.dram_tensor(nm, shape, BF, kind="ExternalOutput").ap()
    _trace(nc, io, nkc)
    nc.compile()
    _BUILT[nkc] = nc
    return nc


def _host_prep(inputs, nkc):
    bf = ml_dtypes.bfloat16
    f32 = np.float32
    KC = nkc * 128
    x = np.asarray(inputs["x"], f32)
    mask = np.asarray(inputs["mask"])

    idx = np.arange(S, dtype=np.float64)
    dd = idx[None, :] - idx[:, None]
    sc = -0.5 * dd * dd
    sc -= sc.max(axis=-1, keepdims=True)
    e = np.exp(sc)
    gauss = (e / e.sum(axis=-1, keepdims=True)).astype(f32)  # [query i, key j]
    gaussT = np.ascontiguousarray(gauss.T)                   # [key t, query s]

    def sbl(a, p=128):  # [C*p, N] -> [p, C*N] (SBUF layout)
        cN = a.shape[0] // p
        return np.ascontiguousarray(
            a.reshape(cN, p, a.shape[1]).transpose(1, 0, 2).reshape(p, -1)
        )

    def pcols(a, p=128):  # [C*p] -> [p, C] per-partition columns
        return np.ascontiguousarray(a.reshape(-1, p).T)

    def thmajor(w):  # [D, D] -> [p, th, k, c] stationary tiles, th-major
        t = w.reshape(KD, 128, KD, 128)           # [k, p, th, c]
        return np.ascontiguousarray(t.transpose(1, 2, 0, 3).reshape(128, -1))

    # LN1 gamma/beta folded into the FFN path (LN1 emits the normalized row):
    #   ff-in  = gamma1*h + beta1  ->  w1' = diag(gamma1) @ w1, b1' = b1 + beta1 @ w1
    #   resid  = gamma1*h (+ beta1 folded into b2')
    gamma1 = np.asarray(inputs["gamma1"], f32)
    beta1 = np.asarray(inputs["beta1"], f32)
    w1 = np.asarray(inputs["w1"], f32)
    w1f = gamma1[:, None] * w1
    b1f = np.asarray(inputs["b1"], f32) + beta1 @ w1
    b2f = np.asarray(inputs["b2"], f32) + beta1
    bias_f = np.concatenate(
        [
            pcols(np.asarray(inputs["bq"], f32) * np.float32(0.125)),
            pcols(np.asarray(inputs["bk"], f32)),
            pcols(b1f),
        ],
        axis=1,
    )
    rows2 = np.concatenate(
        [
            np.asarray(inputs["bv"], f32),
            np.asarray(inputs["b_proj"], f32),
            b2f,
            gamma1,
            beta1,
            np.asarray(inputs["gamma2"], f32),
            np.asarray(inputs["beta2"], f32),
        ]
    ).astype(bf)[None, :]
    shared = {
        "wq_bf": thmajor((np.asarray(inputs["wq"], f32)
                          * np.float32(0.125)).astype(bf)),
        "wk_bf": thmajor(np.asarray(inputs["wk"], f32).astype(bf)),
        "wv_bf": sbl(np.asarray(inputs["wv"], f32).astype(bf)),
        "wproj_bf": sbl(np.asarray(inputs["w_proj"], f32).astype(bf)),
        "w1_bf": sbl(w1f.astype(bf)).reshape(128, KD, Fd),
        "w2_bf": sbl(np.asarray(inputs["w2"], f32).astype(bf)).reshape(128, KF, D),
        "bias_f": bias_f,
        "rows2": rows2,
    }
    in_maps = []
    for b in range(NCORES):
        m = dict(shared)
        xb = np.ascontiguousarray(x[b])
        xT = np.ascontiguousarray(xb.T)
        m["xxT"] = np.concatenate(
            [sbl(xb.astype(bf)), sbl(xT.astype(bf))], axis=1
        )
        valid = np.nonzero(mask[b])[0]
        nv = len(valid)
        idxp = np.zeros(KC, dtype=np.int64)
        idxp[:nv] = valid
        xTc = xT[:, idxp]                       # [768, KC]
        m["xTc"] = sbl(np.ascontiguousarray(xTc).astype(bf))
        gc = np.exp(gaussT[idxp, :]).astype(f32)  # [KC, S] exp-domain
        gc[nv:, :] = 0.0
        m["gexpc"] = sbl(gc.astype(bf), p=128)
        in_maps.append(m)
    return in_maps


def _run(inputs, trace=False, trace_cores=None):
    mask = np.asarray(inputs["mask"])
    maxv = int(mask.astype(np.int64).sum(axis=1).max())
    nkc = 3 if maxv <= 384 else 4
    nc = _build(nkc)
    in_maps = _host_prep(inputs, nkc)
    res = bass_utils.run_bass_kernel_spmd(
        nc, in_maps, core_ids=list(range(NCORES)), trace=trace,
        trace_cores=trace_cores,
    )
    out = np.stack([np.asarray(res.results[b]["out"]) for b in range(NCORES)])
    return out.astype(np.float32), res


def kernel(**inputs):
    return _run(inputs)[0]


# revision 73
# speedup vs baseline: 1.1909x; 1.1909x over previous
"""Trainium2 Bass kernel for a dense transformer block (B=8, S=512, D=768, H=12, Fd=3072).

Sharding: pure data-parallel over batch - one batch element per NeuronCore,
weights replicated, no collectives.

Key design points (v3):
  - T-layout attention (activations kept [feature, seq]) so the PE never
    transposes the attention matrix.
  - Host-side KEY COMPACTION: only the mask==1 key positions (~256 of 512,
    padded to KC=384) participate in scores / exp / attn@V.
  - The gaussian positional bias (+ -30 pad-kill) is ADDED into the scores
    PSUM via an identity-matmul (glog moving operand), so the softmax is a
    single Exp activation per chunk - no elementwise multiplies.
  - Softmax normalization: [v | 1] augmented V gives per-query sums free;
    sum row -> SBUF (DVE copy), reciprocal_approx_fast, gpsimd
    partition_broadcast (proxy ucode library), one DVE multiply.
  - LN1 emits the normalized row via one scalar-engine ACT (per-partition
    scale/bias); gamma1/beta1 are folded into w1/b1/b2 on host.
  - Weights th-major so the first q/k tiles need 1/6 of the weight DMA;
    DMA ring order defers w1/w2 behind the critical lead-in tensors.
  - FFN identical structure to v1 (it ran at 91-96% PE efficiency).

Matmul inputs are bf16 (weights pre-cast on host), PSUM accumulation f32.
"""

import os
import numpy as np
import ml_dtypes

import concourse.bass as bass
import concourse.mybir as mybir
import concourse.tile as tile
from concourse import bacc
from concourse import bass_utils
from concourse.masks import make_identity
from concourse import library_config

_DBG = bool(os.environ.get("KBENCH_DEBUG_DUMP"))

BF = mybir.dt.bfloat16
F32 = mybir.dt.float32
AF = mybir.ActivationFunctionType
OP = mybir.AluOpType

B, S, D, H, Dh, Fd = 8, 512, 768, 12, 64, 3072
NCORES = 8

KD = D // 128      # 6  K-tiles over D
MS = S // 128      # 4  chunks over sequence
KF = Fd // 128     # 24 K-tiles over Fd
NT = 2             # N-tiles over D for natural-layout outputs (2 x 384)
ND = D // NT       # 384
EPS = 1e-12


def _trace(nc, io, nkc):
    with tile.TileContext(nc) as tc:
        _trace_body(nc, tc, io, nkc)


def _trace_body(nc, tc, io, nkc):
    from contextlib import ExitStack

    KC = nkc * 128  # compacted key count

    with ExitStack() as ctx:
        glob = ctx.enter_context(tc.tile_pool(name="glob", bufs=1))

        # ---- constants / small inputs (gpsimd ring: smalls first) ----
        biasf_sb = glob.tile([128, 2 * KD + KF], F32, tag="biasf")
        nc.gpsimd.dma_start(out=biasf_sb, in_=io["bias_f"])
        bq_c = biasf_sb[:, 0:KD]
        bk_c = biasf_sb[:, KD : 2 * KD]
        b1_c = biasf_sb[:, 2 * KD : 2 * KD + KF]

        # biasb and gamma/beta rows packed into one single-partition row
        smalls2 = glob.tile([1, 7 * D], BF, tag="smalls2")
        nc.gpsimd.dma_start(out=smalls2, in_=io["rows2"])
        bv_r = smalls2[:, 0:D]
        bproj_r = smalls2[:, D : 2 * D]
        b2_r = smalls2[:, 2 * D : 3 * D]
        gb_row = smalls2[:, 3 * D : 7 * D]
        gbt = glob.tile([128, 4, D], BF, tag="gbt")
        g1b, be1b, g2b, be2b = gbt[:, 0, :], gbt[:, 1, :], gbt[:, 2, :], gbt[:, 3, :]

        ident_bf = glob.tile([128, 128], BF, tag="ident")
        make_identity(nc, ident_bf)
        ones_bf = glob.tile([1, 512], BF, tag="ones_bf")
        nc.vector.memset(ones_bf, 1.0)
        eps_t = glob.tile([128, 1], F32, tag="eps")
        nc.vector.memset(eps_t, EPS)


        # x and xT share one wide-row tile (single fast DMA)
        xxT_sb = glob.tile([128, MS * D + KD * S], BF, tag="xxT")
        x_sb = xxT_sb[:, 0 : MS * D].rearrange("p (c n) -> p c n", n=D)
        xT_sb = xxT_sb[:, MS * D :].rearrange("p (c s) -> p c s", s=S)
        hT_sb = glob.tile([128, KD, S], BF, tag="hT")
        h1b_sb = glob.tile([128, MS, D], BF, tag="h1b")
        h1T_sb = glob.tile([128, KD, S], BF, tag="h1T")
        w1_sb = glob.tile([128, KD, Fd], BF, tag="w1")
        # w2 at top level so it can stream in during attention
        w2_sb = glob.tile([128, KF, D], BF, tag="w2")

        last_mult = None

        # ================= attention scope =================
        with ExitStack() as attn_ctx:
            attnp = attn_ctx.enter_context(tc.tile_pool(name="attn", bufs=1))
            # sync ring: xTc first (critical)
            xTc_sb = attnp.tile([128, KD, KC], BF, tag="xTc")
            nc.sync.dma_start(
                out=xTc_sb.rearrange("p c s -> p (c s)"), in_=io["xTc"]
            )
            gexpc_sb = attnp.tile([128, nkc, S], BF, tag="gexpc")
            wv_sb = attnp.tile([128, KD, D], BF, tag="wv")
            wp_sb = attnp.tile([128, KD, D], BF, tag="wp")
            qT_sb = attnp.tile([128, KD, S], BF, tag="qT")
            kTc_sb = attnp.tile([128, KD, KC], BF, tag="kTc")
            vc_sb = attnp.tile([128, nkc, H, Dh + 1], BF, tag="vc")

            # ---- q/k projections in a short-lived weight pool ----
            psA_cm = tc.tile_pool(name="psA", bufs=1, space="PSUM")
            psA = psA_cm.__enter__()
            with tc.tile_pool(name="wqwk", bufs=1) as wqwk:
                # gpsimd ring (after smalls): wk th-major halves, gexpc
                wk_sb = wqwk.tile([128, KD, KD, 128], BF, tag="wk")
                nc.gpsimd.dma_start(
                    out=wk_sb[:, 0:3].rearrange("p a b c -> p (a b c)"),
                    in_=io["wk_bf"][:, 0 : 3 * KD * 128],
                )
                nc.gpsimd.dma_start(
                    out=wk_sb[:, 3:6].rearrange("p a b c -> p (a b c)"),
                    in_=io["wk_bf"][:, 3 * KD * 128 : 6 * KD * 128],
                )
                nc.gpsimd.dma_start(
                    out=gexpc_sb.rearrange("p c s -> p (c s)"), in_=io["gexpc"]
                )
                nc.gpsimd.load_library(library_config.proxy)
                nc.gpsimd.partition_broadcast(
                    gbt.rearrange("p c n -> p (c n)"), gb_row
                )
                # sync ring (after xTc): wq halves
                wq_sb = wqwk.tile([128, KD, KD, 128], BF, tag="wq")
                nc.sync.dma_start(
                    out=wq_sb[:, 0:3].rearrange("p a b c -> p (a b c)"),
                    in_=io["wq_bf"][:, 0 : 3 * KD * 128],
                )
                nc.sync.dma_start(
                    out=wq_sb[:, 3:6].rearrange("p a b c -> p (a b c)"),
                    in_=io["wq_bf"][:, 3 * KD * 128 : 6 * KD * 128],
                )
                # scalar ring: wv (for v right after kq), then x+xT, wp
                nc.scalar.dma_start(
                    out=wv_sb.rearrange("p c n -> p (c n)"), in_=io["wv_bf"]
                )
                nc.scalar.dma_start(out=xxT_sb, in_=io["xxT"])
                nc.scalar.dma_start(
                    out=wp_sb.rearrange("p c n -> p (c n)"), in_=io["wproj_bf"]
                )
                nc.vector.memset(vc_sb[:, :, :, Dh : Dh + 1], 1.0)

                def kq_tile(th, which):
                    if which == "k":
                        w, xx, n, dst, bias = wk_sb, xTc_sb, KC, kTc_sb, bk_c
                    else:
                        w, xx, n, dst, bias = wq_sb, xT_sb, S, qT_sb, bq_c
                    ps = psA.tile([128, 512], F32, tag="acc", bufs=3,
                                  name="ps_kq")
                    for k in range(KD):
                        nc.tensor.matmul(
                            ps[:, 0:n], w[:, th, k, :], xx[:, k, :],
                            start=(k == 0), stop=(k == KD - 1),
                        )
                    # PSUM->SBUF move + per-partition bias on DVE, keeping
                    # the scalar engine free for the exp stream
                    return nc.vector.tensor_scalar_add(
                        out=dst[:, th, :], in0=ps[:, 0:n],
                        scalar1=bias[:, th : th + 1],
                    )

                for th in range(3):
                    kq_tile(th, "k")
                for th in range(3):
                    kq_tile(th, "q")
                for th in range(3, KD):
                    kq_tile(th, "k")
                for th in range(3, KD):
                    kq_tile(th, "q")

            def v_tiles(n):
                for c in range(nkc):
                    ps = psA.tile([128, 512], F32, tag="acc", bufs=3, name="ps_v")
                    for k in range(KD):
                        nc.tensor.matmul(
                            ps[:, 0:ND], xTc_sb[:, k, 128 * c : 128 * (c + 1)],
                            wv_sb[:, k, ND * n : ND * (n + 1)],
                            start=(k == 0), stop=False,
                        )
                    nc.tensor.matmul(
                        ps[:, 0:ND], ones_bf[:, 0:128],
                        bv_r[:, ND * n : ND * (n + 1)],
                        start=False, stop=True,
                    )
                    nc.vector.tensor_copy(
                        out=vc_sb[:, c, 6 * n : 6 * (n + 1), 0:Dh],
                        in_=ps[:, 0:ND].rearrange("p (h d) -> p h d", d=Dh),
                    )

            # w1 / w2 transfers ride the rings behind the critical lead-in
            nc.scalar.dma_start(
                out=w1_sb[:, 0:3, :].rearrange("p c n -> p (c n)"),
                in_=io["w1_bf"][:, 0:3, :].rearrange("p c n -> p (c n)"),
            )
            nc.gpsimd.dma_start(
                out=w1_sb[:, 3:6, :].rearrange("p c n -> p (c n)"),
                in_=io["w1_bf"][:, 3:6, :].rearrange("p c n -> p (c n)"),
            )
            nc.scalar.dma_start(
                out=w2_sb[:, 0:12, :].rearrange("p c n -> p (c n)"),
                in_=io["w2_bf"][:, 0:12, :].rearrange("p c n -> p (c n)"),
            )
            nc.gpsimd.dma_start(
                out=w2_sb[:, 12:24, :].rearrange("p c n -> p (c n)"),
                in_=io["w2_bf"][:, 12:24, :].rearrange("p c n -> p (c n)"),
            )
            v_tiles(0)
            v_tiles(1)
            psA_cm.__exit__(None, None, None)

            if _DBG:
                nc.scalar.dma_start(
                    out=io["dbg_qT"], in_=qT_sb.rearrange("p a b -> p (a b)")
                )
                nc.scalar.dma_start(
                    out=io["dbg_kTc"], in_=kTc_sb.rearrange("p a b -> p (a b)")
                )

            # ---- heads scope ----
            with tc.tile_pool(name="headsp", bufs=1) as hp, \
                 tc.tile_pool(name="psS", bufs=1, space="PSUM") as psS, \
                 tc.tile_pool(name="psH", bufs=1, space="PSUM") as psH:

                def head_front(h):
                    th, off = h // 2, (h % 2) * 64
                    qh = qT_sb[off : off + 64, th, :]
                    exs = []
                    for c in range(nkc):
                        ps_sc = psS.tile([128, 512], F32, tag="sc", bufs=4,
                                         name="ps_sc")
                        nc.tensor.matmul(
                            ps_sc, kTc_sb[off : off + 64, th,
                                          128 * c : 128 * (c + 1)],
                            qh, start=True, stop=True,
                        )
                        ex = hp.tile([128, 512], BF, tag="ex", bufs=9, name="ex")
                        nc.scalar.activation(out=ex, in_=ps_sc, func=AF.Exp)
                        nc.vector.tensor_tensor(
                            out=ex, in0=ex, in1=gexpc_sb[:, c, :], op=OP.mult
                        )
                        exs.append(ex)
                    ps_h = psH.tile([Dh + 1, 512], F32, tag="hT", bufs=4,
                                    name="ps_h")
                    for c in range(nkc):
                        nc.tensor.matmul(
                            ps_h, vc_sb[:, c, h, :], exs[c],
                            start=(c == 0), stop=(c == nkc - 1),
                        )
                    return ps_h

                def head_tail(h, ps_h):
                    th, off = h // 2, (h % 2) * 64
                    srow = hp.tile([1, 512], F32, tag="srow", bufs=3, name="srow")
                    nc.scalar.copy(out=srow, in_=ps_h[Dh : Dh + 1, :])
                    rec = hp.tile([1, 512], F32, tag="rec", bufs=3, name="rec")
                    nc.vector.reciprocal_approx_fast(out=rec, in_=srow)
                    bca = hp.tile([64, 512], F32, tag="bca", bufs=3, name="bca")
                    nc.gpsimd.partition_broadcast(bca, rec)
                    return nc.vector.tensor_tensor(
                        out=hT_sb[off : off + 64, th, :], in0=ps_h[0:Dh, :],
                        in1=bca, op=OP.mult,
                    )

                pending = None
                for h in range(H):
                    ps_h = head_front(h)
                    if pending is not None:
                        head_tail(*pending)
                    pending = (h, ps_h)
                last_mult = head_tail(*pending)

            # ---- proj + residual + LN1 + h1 transpose ----
            with tc.tile_pool(name="psP", bufs=1, space="PSUM") as psP, \
                 tc.tile_pool(name="psT", bufs=1, space="PSUM") as psT:
                for m in range(MS):
                    pss = []
                    for n in range(NT):
                        ps = psP.tile([128, 512], F32, tag="pacc", bufs=3,
                                      name="ps_pr")
                        for k in range(KD):
                            nc.tensor.matmul(
                                ps[:, 0:ND],
                                hT_sb[:, k, 128 * m : 128 * (m + 1)],
                                wp_sb[:, k, ND * n : ND * (n + 1)],
                                start=(k == 0), stop=False,
                            )
                        nc.tensor.matmul(
                            ps[:, 0:ND], ones_bf[:, 0:128],
                            bproj_r[:, ND * n : ND * (n + 1)],
                            start=False, stop=True,
                        )
                        pss.append(ps)
                    row = glob.tile([128, D], F32, tag="rowtmp", bufs=2, name="row")
                    nc.vector.tensor_tensor(
                        out=row[:, 0:ND], in0=pss[0][:, 0:ND],
                        in1=x_sb[:, m, 0:ND], op=OP.add,
                    )
                    nc.vector.tensor_tensor(
                        out=row[:, ND:D], in0=pss[1][:, 0:ND],
                        in1=x_sb[:, m, ND:D], op=OP.add,
                    )
                    _layernorm(nc, glob, row, g1b, be1b, eps_t, h1b_sb[:, m, :],
                               fold=True)
                    for f in range(KD):
                        ps_t = psT.tile([128, 128], BF, tag="tr", bufs=2,
                                        name="ps_t")
                        nc.tensor.transpose(
                            ps_t, h1b_sb[:, m, 128 * f : 128 * (f + 1)], ident_bf
                        )
                        if f % 2 == 0:
                            nc.scalar.copy(
                                out=h1T_sb[:, f, 128 * m : 128 * (m + 1)], in_=ps_t
                            )
                        else:
                            nc.vector.tensor_copy(
                                out=h1T_sb[:, f, 128 * m : 128 * (m + 1)], in_=ps_t
                            )

        if _DBG:
            nc.scalar.dma_start(
                out=io["dbg_hT"], in_=hT_sb.rearrange("p a b -> p (a b)")
            )
            nc.scalar.dma_start(
                out=io["dbg_h1b"], in_=h1b_sb.rearrange("p a b -> p (a b)")
            )
            nc.scalar.dma_start(
                out=io["dbg_h1T"], in_=h1T_sb.rearrange("p a b -> p (a b)")
            )

        # ================= FFN scope =================
        with tc.tile_pool(name="ffnp", bufs=1) as ffnp, \
             tc.tile_pool(name="psF", bufs=1, space="PSUM") as psF:
            ff1T_sb = ffnp.tile([128, KF, S], BF, tag="ff1T")
            for fm in range(KF):
                ps = psF.tile([128, 512], F32, tag="facc", bufs=8, name="ps_f1")
                for k in range(KD):
                    nc.tensor.matmul(
                        ps, w1_sb[:, k, 128 * fm : 128 * (fm + 1)], h1T_sb[:, k, :],
                        start=(k == 0), stop=(k == KD - 1),
                    )
                nc.scalar.activation(
                    out=ff1T_sb[:, fm, :], in_=ps, func=AF.Gelu,
                    bias=b1_c[:, fm : fm + 1], scale=1.0,
                )

            out_engines = (nc.sync, nc.gpsimd, nc.scalar)
            for m in range(MS):
                pss = []
                for n in range(NT):
                    ps = psF.tile([128, 512], F32, tag="facc", bufs=8, name="ps_f2")
                    for k in range(KF):
                        nc.tensor.matmul(
                            ps[:, 0:ND],
                            ff1T_sb[:, k, 128 * m : 128 * (m + 1)],
                            w2_sb[:, k, ND * n : ND * (n + 1)],
                            start=(k == 0), stop=False,
                        )
                    nc.tensor.matmul(
                        ps[:, 0:ND], ones_bf[:, 0:128],
                        b2_r[:, ND * n : ND * (n + 1)],
                        start=False, stop=True,
                    )
                    pss.append(ps)
                # residual = gamma1 * h1_hat (+ beta1 folded into b2 row)
                tmp = glob.tile([128, D], F32, tag="restmp", bufs=2, name="tmp")
                nc.vector.tensor_tensor(
                    out=tmp, in0=h1b_sb[:, m, :], in1=g1b, op=OP.mult
                )
                row = glob.tile([128, D], F32, tag="rowtmp", bufs=2, name="row2")
                nc.vector.tensor_tensor(
                    out=row[:, 0:ND], in0=pss[0][:, 0:ND],
                    in1=tmp[:, 0:ND], op=OP.add,
                )
                nc.vector.tensor_tensor(
                    out=row[:, ND:D], in0=pss[1][:, 0:ND],
                    in1=tmp[:, ND:D], op=OP.add,
                )
                outrow = glob.tile([128, D], BF, tag="outrow", bufs=2,
                                   name="outrow")
                _layernorm(nc, glob, row, g2b, be2b, eps_t, outrow)
                if m < MS - 1:
                    out_engines[m].dma_start(
                        out=io["out"][128 * m : 128 * (m + 1), :], in_=outrow
                    )
                else:
                    nc.sync.dma_start(
                        out=io["out"][128 * m : 128 * (m + 1), 0:ND],
                        in_=outrow[:, 0:ND],
                    )
                    nc.scalar.dma_start(
                        out=io["out"][128 * m : 128 * (m + 1), ND:D],
                        in_=outrow[:, ND:D],
                    )


def _layernorm(nc, pool, row, gamma_b, beta_b, eps_t, out_ap, fold=False):
    st = pool.tile([128, 3, 6], F32, tag="st", bufs=2, name="st")
    for g in range(3):
        nc.vector.bn_stats(out=st[:, g, :], in_=row[:, 256 * g : 256 * (g + 1)])
    mv = pool.tile([128, 2], F32, tag="mv", bufs=2, name="mv")
    nc.vector.bn_aggr(out=mv, in_=st)
    sd = pool.tile([128, 1], F32, tag="sd", bufs=2, name="sd")
    nc.scalar.activation(out=sd, in_=mv[:, 1:2], func=AF.Sqrt, bias=eps_t, scale=1.0)
    rs = pool.tile([128, 1], F32, tag="rs", bufs=2, name="rs")
    nc.vector.reciprocal(rs, sd)
    if fold:
        # gamma/beta folded into downstream weights on host: emit the
        # normalized row via one ACT (per-partition scale rstd, bias -mu*rstd)
        nb = pool.tile([128, 1], F32, tag="nb", bufs=2, name="nb")
        nc.vector.scalar_tensor_tensor(
            out=nb, in0=mv[:, 0:1], scalar=-1.0, in1=rs,
            op0=OP.mult, op1=OP.mult,
        )
        nc.scalar.activation(
            out=out_ap, in_=row, func=AF.Identity, bias=nb, scale=rs
        )
        return
    # in-place: row = (row - mean) * gamma ; out = row * rstd + beta
    nc.vector.scalar_tensor_tensor(
        out=row, in0=row, scalar=mv[:, 0:1], in1=gamma_b,
        op0=OP.subtract, op1=OP.mult,
    )
    nc.vector.scalar_tensor_tensor(
        out=out_ap, in0=row, scalar=rs, in1=beta_b,
        op0=OP.mult, op1=OP.add,
    )


def _specs(nkc):
    KC = nkc * 128
    return [
        ("xxT", [128, MS * D + KD * S], BF),   # x | xT packed (wide rows)
        ("xTc", [128, KD * KC], BF),
        ("gexpc", [128, nkc * S], BF),
        ("wq_bf", [128, KD * KD * 128], BF),   # th-major [p, th, k, c]
        ("wk_bf", [128, KD * KD * 128], BF),   # th-major
        ("wv_bf", [128, KD * D], BF),
        ("wproj_bf", [128, KD * D], BF),
        ("w1_bf", [128, KD, Fd], BF),
        ("w2_bf", [128, KF, D], BF),
        ("bias_f", [128, 2 * KD + KF], F32),   # bq*0.125 | bk | b1' pcols
        ("rows2", [1, 7 * D], BF),             # bv|bproj|b2'|gamma1|beta1|gamma2|beta2
    ]


_BUILT = {}


def _build(nkc):
    if nkc in _BUILT:
        return _BUILT[nkc]
    nc = bacc.Bacc("TRN2", target_bir_lowering=False, debug=False,
                   enable_asserts=False, num_devices=NCORES)
    io = {}
    for name, shape, dt in _specs(nkc):
        io[name] = nc.dram_tensor(name, shape, dt, kind="ExternalInput").ap()
    io["out"] = nc.dram_tensor("out", [S, D], BF, kind="ExternalOutput").ap()
    io["bscratch"] = nc.dram_tensor("bscratch", [3, 4, 512], F32,
                                    kind="Internal").ap()
    if _DBG:
        for nm, shape in [
            ("dbg_qT", [128, KD * S]), ("dbg_kTc", [128, KD * nkc * 128]),
            ("dbg_hT", [128, KD * S]),
            ("dbg_h1b", [128, MS * D]), ("dbg_h1T", [128, KD * S]),
        ]:
            io[nm] = nc.dram_tensor(nm, shape, BF, kind="ExternalOutput").ap()
    _trace(nc, io, nkc)
    nc.compile()
    _BUILT[nkc] = nc
    return nc


def _host_prep(inputs, nkc):
    bf = ml_dtypes.bfloat16
    f32 = np.float32
    KC = nkc * 128
    x = np.asarray(inputs["x"], f32)
    mask = np.asarray(inputs["mask"])

    idx = np.arange(S, dtype=np.float64)
    dd = idx[None, :] - idx[:, None]
    sc = -0.5 * dd * dd
    sc -= sc.max(axis=-1, keepdims=True)
    e = np.exp(sc)
    gauss = (e / e.sum(axis=-1, keepdims=True)).astype(f32)  # [query i, key j]
    gaussT = np.ascontiguousarray(gauss.T)                   # [key t, query s]

    def sbl(a, p=128):  # [C*p, N] -> [p, C*N] (SBUF layout)
        cN = a.shape[0] // p
        return np.ascontiguousarray(
            a.reshape(cN, p, a.shape[1]).transpose(1, 0, 2).reshape(p, -1)
        )

    def pcols(a, p=128):  # [C*p] -> [p, C] per-partition columns
        return np.ascontiguousarray(a.reshape(-1, p).T)

    def thmajor(w):  # [D, D] -> [p, th, k, c] stationary tiles, th-major
        t = w.reshape(KD, 128, KD, 128)           # [k, p, th, c]
        return np.ascontiguousarray(t.transpose(1, 2, 0, 3).reshape(128, -1))

    # LN1 gamma/beta folded into the FFN path (LN1 emits the normalized row):
    #   ff-in  = gamma1*h + beta1  ->  w1' = diag(gamma1) @ w1, b1' = b1 + beta1 @ w1
    #   resid  = gamma1*h (+ beta1 folded into b2')
    gamma1 = np.asarray(inputs["gamma1"], f32)
    beta1 = np.asarray(inputs["beta1"], f32)
    w1 = np.asarray(inputs["w1"], f32)
    w1f = gamma1[:, None] * w1
    b1f = np.asarray(inputs["b1"], f32) + beta1 @ w1
    b2f = np.asarray(inputs["b2"], f32) + beta1
    bias_f = np.concatenate(
        [
            pcols(np.asarray(inputs["bq"], f32) * np.float32(0.125)),
            pcols(np.asarray(inputs["bk"], f32)),
            pcols(b1f),
        ],
        axis=1,
    )
    rows2 = np.concatenate(
        [
            np.asarray(inputs["bv"], f32),
            np.asarray(inputs["b_proj"], f32),
            b2f,
            gamma1,
            beta1,
            np.asarray(inputs["gamma2"], f32),
            np.asarray(inputs["beta2"], f32),
        ]
    ).astype(bf)[None, :]
    shared = {
        "wq_bf": thmajor((np.asarray(inputs["wq"], f32)
                          * np.float32(0.125)).astype(bf)),
        "wk_bf": thmajor(np.asarray(inputs["wk"], f32).astype(bf)),
        "wv_bf": sbl(np.asarray(inputs["wv"], f32).astype(bf)),
        "wproj_bf": sbl(np.asarray(inputs["w_proj"], f32).astype(bf)),
        "w1_bf": sbl(w1f.astype(bf)).reshape(128, KD, Fd),
        "w2_bf": sbl(np.asarray(inputs["w2"], f32).astype(bf)).reshape(128, KF, D),
        "bias_f": bias_f,
        "rows2": rows2,
    }
    in_maps = []
    for b in range(NCORES):
        m = dict(shared)
        xb = np.ascontiguousarray(x[b])
        xT = np.ascontiguousarray(xb.T)
        m["xxT"] = np.concatenate(
            [sbl(xb.astype(bf)), sbl(xT.astype(bf))], axis=1
        )
        valid = np.nonzero(mask[b])[0]
        nv = len(valid)
        idxp = np.zeros(KC, dtype=np.int64)
        idxp[:nv] = valid
        xTc = xT[:, idxp]                       # [768, KC]
        m["xTc"] = sbl(np.ascontiguousarray(xTc).astype(bf))
        gc = np.exp(gaussT[idxp, :]).astype(f32)  # [KC, S] exp-domain
        gc[nv:, :] = 0.0
        m["gexpc"] = sbl(gc.astype(bf), p=128)
        in_maps.append(m)
    return in_maps


def _run(inputs, trace=False, trace_cores=None):
    mask = np.asarray(inputs["mask"])
    maxv = int(mask.astype(np.int64).sum(axis=1).max())
    nkc = 3 if maxv <= 384 else 4
    nc = _build(nkc)
    in_maps = _host_prep(inputs, nkc)
    res = bass_utils.run_bass_kernel_spmd(
        nc, in_maps, core_ids=list(range(NCORES)), trace=trace,
        trace_cores=trace_cores,
    )
    out = np.stack([np.asarray(res.results[b]["out"]) for b in range(NCORES)])
    return out.astype(np.float32), res


def kernel(**inputs):
    return _run(inputs)[0]
